# revision 1
# baseline (speedup 1.0000x reference)
"""GCN embedding network kernel for Trainium2, sharded across 8 NeuronCores.

Math (derived from the reference GCN):
    A in {0,1}^{NxN};  deg[j] = colsum(A)[j] + 1;  dinv = 1/sqrt(deg)
    y  = x @ W1;  y'[i] = dinv[i] * y[i]
    z[j] = sum_i A[i,j] y'[i] + y'[j]              (Ahat^T aggregation + self loop)
    h[j] = relu(dinv[j] * z[j] + b1)
    u[i] = sum_j A[i,j] dinv[j] + dinv[i]
    r[i] = dinv[i] * u[i]                          (row sums of Ahat)
    out  = (sum_i r[i] h[i]) @ W2 + N * b2         -> [1, F_OUT]

Sharding: rows of A and x are sharded across 8 cores (2048 rows each).
Pass 1 streams the row block computing per-core column-sum partials (and,
in fp8 mode, writes an exact fp8 copy of the 0/1-valued A to DRAM scratch);
an AllReduce (full deg, for u) and a ReduceScatter (own-block deg) follow.
Pass 2 streams the block again computing z^T partials on the PE
(lhsT = y', 3-term fp8 split in fp8 mode) and u on the vector engine; a
ReduceScatter of z^T gives each core the aggregation for its own node
block.  The final weighted row-sum s_p = sum_{i in block} r[i] h[i] is a
[16] vector per core; the host adds the 8 partials and applies the tiny
W2/b2 head.
"""

import numpy as np

import concourse.bass as bass
import concourse.bacc as bacc
import concourse.mybir as mybir
import concourse.tile as tile
from concourse.bass_utils import run_bass_kernel_spmd

# Problem constants (hardcoded per harness contract).
N = 16384
F_IN = 64
HID = 16
F_OUT = 32
NCORES = 8

FP = mybir.dt.float32
FP8 = mybir.dt.float8e4

AF = mybir.ActivationFunctionType
ALU = mybir.AluOpType


def build_gcn(n=N, ncores=NCORES, use_fp8=True, stop_after=None,
              skip_u=False, skip_ztrs=False, skip_zmm=False,
              skip_ztout=False, u_reduce_act=True):
    """Build the SPMD Bass program. Returns the compiled Bacc object.

    stop_after: debug knob - one of None, "p1", "dinv", "yprep", "p2";
    truncated programs fill s_out from whatever is available.
    """
    R = n // ncores            # rows per core == columns per j-block
    IT = R // 128              # 128-row i-tiles per core
    JW = R                     # j-macro width (block-aligned)
    NCH = max(1, JW // 512)    # psum chunks per j-macro
    CW = JW // NCH             # chunk width (<= 512)
    NJ = ncores                # j-macros
    groups = [list(range(ncores))]
    ADT = FP8 if use_fp8 else FP
    _stages = [None, "p1", "dinv", "yprep", "p2"]
    _lvl = 99 if stop_after is None else _stages.index(stop_after)

    nc = bacc.Bacc("TRN2", target_bir_lowering=False, debug=False,
                   num_devices=ncores)

    a_t = nc.dram_tensor("A_blk", [R, n], FP, kind="ExternalInput")
    x_t = nc.dram_tensor("x_blk", [R, F_IN], FP, kind="ExternalInput")
    w1_t = nc.dram_tensor("W1", [F_IN, HID], FP, kind="ExternalInput")
    b1_t = nc.dram_tensor("b1", [HID], FP, kind="ExternalInput")
    s_t = nc.dram_tensor("s_out", [HID, 1], FP, kind="ExternalOutput")

    ident_np = np.eye(128, dtype=np.float32)
    ident_t = nc.inline_tensor(ident_np, name="ident")

    with tile.TileContext(nc) as tc:
        with tc.tile_pool(name="glob", bufs=1) as g, \
             tc.tile_pool(name="dram", bufs=1, space="DRAM") as dram:
            # ---- persistent tiles ----
            ident_sb = g.tile([128, 128], FP)
            ones_sb = g.tile([128, 1], ADT)
            w1_sb = g.tile([F_IN, HID], FP)
            b1_sb = g.tile([HID, 1], FP)
            x3 = g.tile([128, IT, F_IN], FP)
            yp3 = g.tile([128, IT, HID], FP)
            ypT_sb = g.tile([HID, R], FP)
            u_acc = g.tile([128, IT], FP)
            dinv_blk = g.tile([128, IT], FP)
            dinv_rep = g.tile([128, n], FP)
            dinv_own = g.tile([HID, JW], FP)
            hT_sb = g.tile([HID, R], FP)
            r_sb = g.tile([128, IT], FP)
            s_sb = g.tile([HID, 1], FP)
            if use_fp8:
                # 3-term fp8 split of y' at M-offsets 0/32/64 so the psum
                # folds read at 32-aligned base partitions; gaps are zero.
                yp83 = g.tile([128, IT, 96], FP8)
                tmpa = g.tile([128, HID], FP)
                tmpb = g.tile([128, HID], FP)

            # ---- DRAM bounce buffers ----
            csum_in = dram.tile([n], FP)
            csum_ar = dram.tile([n], FP, addr_space="Shared")
            deg_rs = dram.tile([JW], FP)
            zt_in = dram.tile([ncores, HID, JW], FP)
            zt_rs = dram.tile([HID, JW], FP)
            if use_fp8:
                scr8 = dram.tile([NJ, 128, IT, JW], FP8)

            nc.sync.dma_start(ident_sb[:, :], ident_t.ap())
            nc.vector.memset(ones_sb[:, :], 1.0)
            nc.vector.memset(u_acc[:, :], 0.0)
            if use_fp8:
                nc.vector.memset(yp83[:, :, :], 0.0)
            nc.sync.dma_start(w1_sb[:, :], w1_t.ap())
            nc.sync.dma_start(b1_sb[:, :],
                              b1_t.ap().rearrange("(p f) -> p f", f=1))
            nc.sync.dma_start(
                x3[:, :, :], x_t.ap().rearrange("(it p) c -> p it c", p=128))

            # ================= pass 1: column sums (+ fp8 cast) ============
            with tc.tile_pool(name="p1", bufs=3) as p1, \
                 tc.tile_pool(name="p1ps", bufs=2, space="PSUM") as p1ps:
                for jm in range(NJ):
                    cs_ps = p1ps.tile([1, JW], FP, name=f"cs_ps_{jm}",
                                      tag="cs_ps")
                    for it in range(IT):
                        a_tile = p1.tile([128, JW], FP, name=f"a1_{jm}_{it}",
                                         tag="a1")
                        nc.sync.dma_start(
                            a_tile[:, :],
                            a_t.ap()[it * 128:(it + 1) * 128,
                                     jm * JW:(jm + 1) * JW])
                        if use_fp8:
                            a8_sb = p1.tile([128, JW], FP8,
                                            name=f"a8_{jm}_{it}", tag="a8")
                            nc.vector.tensor_copy(a8_sb[:, :], a_tile[:, :])
                            cs_src = a8_sb
                            nc.sync.dma_start(scr8[jm, :, it, :],
                                              a8_sb[:, :])
                        else:
                            cs_src = a_tile
                        for c in range(NCH):
                            nc.tensor.matmul(
                                cs_ps[0:1, c * CW:(c + 1) * CW],
                                ones_sb[:, :],
                                cs_src[:, c * CW:(c + 1) * CW],
                                start=(it == 0), stop=(it == IT - 1))
                    cs_tmp = p1.tile([1, JW], FP, name=f"cs_tmp_{jm}",
                                     tag="cs_tmp", bufs=2)
                    nc.vector.tensor_copy(cs_tmp[0:1, :], cs_ps[0:1, :])
                    nc.sync.dma_start(
                        csum_in.rearrange("(g w) -> g w",
                                          g=ncores)[jm:jm + 1, :],
                        cs_tmp[0:1, :])

            nc.gpsimd.collective_compute(
                "AllReduce", ALU.add, replica_groups=groups,
                ins=[csum_in.opt()], outs=[csum_ar.opt()])
            nc.gpsimd.collective_compute(
                "ReduceScatter", ALU.add, replica_groups=groups,
                ins=[csum_in.opt()], outs=[deg_rs.opt()])

            # ================= dinv preparation ============================
            if _lvl >= 2:
                # dinv_rep[p, j] = 1/sqrt(deg[j] + 1) replicated on 128 partitions
                nc.sync.dma_start(
                    dinv_rep[:, :],
                    bass.AP(csum_ar.tensor, 0, [[0, 128], [1, n]]))
                nc.scalar.activation(dinv_rep[:, :], dinv_rep[:, :], AF.Sqrt,
                                     bias=1.0)
                nc.vector.reciprocal(dinv_rep[:, :], dinv_rep[:, :])
                # dinv_own[f, jj] over own block, replicated on HID partitions
                nc.sync.dma_start(
                    dinv_own[:, :],
                    bass.AP(deg_rs.tensor, 0, [[0, HID], [1, JW]]))
                nc.scalar.activation(dinv_own[:, :], dinv_own[:, :], AF.Sqrt,
                                     bias=1.0)
                nc.vector.reciprocal(dinv_own[:, :], dinv_own[:, :])
                # dinv_blk[p, it] = dinv of own row (it*128 + p)
                nc.sync.dma_start(
                    dinv_blk[:, :],
                    bass.AP(deg_rs.tensor, 0, [[1, 128], [128, IT]]))
                nc.scalar.activation(dinv_blk[:, :], dinv_blk[:, :], AF.Sqrt,
                                     bias=1.0)
                nc.vector.reciprocal(dinv_blk[:, :], dinv_blk[:, :])

            # ================= y' = dinv * (x @ W1) ========================
            if _lvl >= 3:
                with tc.tile_pool(name="yb", bufs=2) as yb, \
                     tc.tile_pool(name="ybps", bufs=2, space="PSUM") as ybps:
                    for it in range(IT):
                        xt_ps = ybps.tile([F_IN, 128], FP, name=f"xt_ps_{it}",
                                          tag="xt_ps")
                        nc.tensor.transpose(xt_ps[:, :], x3[:, it, :],
                                            ident_sb[:, :])
                        xt_sb = yb.tile([F_IN, 128], FP, name=f"xt_sb_{it}",
                                        tag="xt_sb")
                        nc.vector.tensor_copy(xt_sb[:, :], xt_ps[:, :])
                        y_ps = ybps.tile([128, HID], FP, name=f"y_ps_{it}",
                                         tag="y_ps")
                        nc.tensor.matmul(y_ps[:, :], xt_sb[:, :], w1_sb[:, :],
                                         start=True, stop=True)
                        nc.vector.tensor_scalar_mul(
                            yp3[:, it, :], y_ps[:, :], dinv_blk[:, it:it + 1])
                        ypt_ps = ybps.tile([HID, 128], FP, name=f"ypt_ps_{it}",
                                           tag="ypt_ps")
                        nc.tensor.transpose(ypt_ps[:, :], yp3[:, it, :],
                                            ident_sb[:, :])
                        nc.vector.tensor_copy(
                            ypT_sb[:, it * 128:(it + 1) * 128], ypt_ps[:, :])
                        if use_fp8:
                            # 3-term fp8 split of y' (A is exact in fp8;
                            # hi+lo+lo2 carries ~12 extra mantissa bits)
                            nc.vector.tensor_copy(yp83[:, it, 0:HID],
                                                  yp3[:, it, :])
                            nc.vector.tensor_sub(tmpa[:, :], yp3[:, it, :],
                                                 yp83[:, it, 0:HID])
                            nc.vector.tensor_copy(yp83[:, it, 32:32 + HID],
                                                  tmpa[:, :])
                            nc.vector.tensor_sub(tmpb[:, :], tmpa[:, :],
                                                 yp83[:, it, 32:32 + HID])
                            nc.vector.tensor_copy(yp83[:, it, 64:64 + HID],
                                                  tmpb[:, :])

            # ================= pass 2 =====================================
            if _lvl >= 4:
                MOUT = 96 if use_fp8 else HID
                with tc.tile_pool(name="p2", bufs=(2 if use_fp8 else 3)) as p2, \
                     tc.tile_pool(name="p2s", bufs=1) as p2s, \
                     tc.tile_pool(name="p2ps", bufs=2, space="PSUM") as p2ps:
                    for jm in range(NJ):
                        zt_ps = p2ps.tile([MOUT, JW], FP, name=f"zt_ps_{jm}",
                                          tag="zt_ps")
                        if use_fp8:
                            a8 = p2.tile([128, IT, JW], FP8, name=f"a8_{jm}",
                                         tag="a8")
                            nc.sync.dma_start(a8[:, :, :], scr8[jm, :, :, :])
                        for it in range(IT):
                            if use_fp8:
                                rhs_full = a8[:, it, :]
                                lhsT = yp83[:, it, :]
                            else:
                                a_tile = p2.tile([128, JW], FP,
                                                 name=f"a2_{jm}_{it}", tag="a2")
                                nc.sync.dma_start(
                                    a_tile[:, :],
                                    a_t.ap()[it * 128:(it + 1) * 128,
                                             jm * JW:(jm + 1) * JW])
                                rhs_full = a_tile[:, :]
                                lhsT = yp3[:, it, :]
                            if not skip_zmm:
                                for c in range(NCH):
                                    nc.tensor.matmul(
                                        zt_ps[:, c * CW:(c + 1) * CW],
                                        lhsT,
                                        rhs_full[:, c * CW:(c + 1) * CW],
                                        start=(it == 0), stop=(it == IT - 1))
                            if skip_u:
                                continue
                            # u partial: row-wise sum of A * dinv (DVE mul,
                            # then free-dim reduce on ACT or DVE)
                            prod = p2s.tile([128, JW], FP,
                                            name=f"prod_{jm}_{it}",
                                            tag="prod", bufs=2)
                            nc.vector.tensor_mul(
                                prod[:, :], rhs_full,
                                dinv_rep[:, jm * JW:(jm + 1) * JW])
                            red = p2s.tile([128, 1], FP,
                                           name=f"red_{jm}_{it}",
                                           tag="red", bufs=2)
                            if u_reduce_act:
                                trash = p2s.tile([128, JW], FP,
                                                 name=f"trash_{jm}_{it}",
                                                 tag="trash", bufs=2)
                                nc.scalar.activation(
                                    trash[:, :], prod[:, :], AF.Copy,
                                    accum_out=red[:, 0:1])
                            else:
                                nc.vector.tensor_reduce(
                                    red[:, 0:1], prod[:, :],
                                    axis=mybir.AxisListType.X, op=ALU.add)
                            nc.vector.tensor_add(
                                u_acc[:, it:it + 1], u_acc[:, it:it + 1],
                                red[:, 0:1])
                        if skip_ztout or skip_zmm:
                            continue
                        zt_tmp = p2s.tile([HID, JW], FP, name=f"zt_tmp_{jm}",
                                          tag="zt_tmp", bufs=2)
                        nc.vector.tensor_copy(zt_tmp[:, :], zt_ps[0:HID, :])
                        if use_fp8:
                            nc.vector.tensor_add(
                                zt_tmp[:, :], zt_tmp[:, :],
                                zt_ps[32:32 + HID, :])
                            nc.vector.tensor_add(
                                zt_tmp[:, :], zt_tmp[:, :],
                                zt_ps[64:64 + HID, :])
                        nc.sync.dma_start(zt_in[jm, :, :], zt_tmp[:, :])

                if not skip_ztrs:
                    nc.gpsimd.collective_compute(
                        "ReduceScatter", ALU.add, replica_groups=groups,
                        ins=[zt_in.opt()], outs=[zt_rs.opt()])

            # ================= h, r, s_p ===================================
            if _lvl >= 99:
                with tc.tile_pool(name="fin", bufs=2) as fin, \
                     tc.tile_pool(name="fps", bufs=2, space="PSUM") as fps, \
                     tc.tile_pool(name="sps", bufs=1, space="PSUM") as sps:
                    nc.sync.dma_start(hT_sb[:, :], zt_rs[:, :])
                    # z += y' (self loop), * dinv, relu(. + b1)
                    nc.vector.tensor_add(hT_sb[:, :], hT_sb[:, :], ypT_sb[:, :])
                    nc.vector.tensor_mul(hT_sb[:, :], hT_sb[:, :],
                                         dinv_own[:, :])
                    nc.scalar.activation(hT_sb[:, :], hT_sb[:, :], AF.Relu,
                                         bias=b1_sb[:, 0:1])
                    # r = dinv * (u + dinv)
                    nc.vector.tensor_add(r_sb[:, :], u_acc[:, :],
                                         dinv_blk[:, :])
                    nc.vector.tensor_mul(r_sb[:, :], r_sb[:, :],
                                         dinv_blk[:, :])
                    s_ps = sps.tile([HID, 1], FP)
                    for it in range(IT):
                        h_ps = fps.tile([128, HID], FP, name=f"h_ps_{it}",
                                        tag="h_ps")
                        nc.tensor.transpose(
                            h_ps[:, :], hT_sb[:, it * 128:(it + 1) * 128],
                            ident_sb[0:HID, 0:HID])
                        h_sb = fin.tile([128, HID], FP, name=f"h_sb_{it}",
                                        tag="h_sb")
                        nc.vector.tensor_copy(h_sb[:, :], h_ps[:, :])
                        nc.tensor.matmul(s_ps[:, :], h_sb[:, :],
                                         r_sb[:, it:it + 1],
                                         start=(it == 0), stop=(it == IT - 1))
                    nc.vector.tensor_copy(s_sb[:, :], s_ps[:, :])
                    nc.sync.dma_start(s_t.ap(), s_sb[:, :])

    nc.compile()
    return nc


_NC_CACHE = {}


def _get_nc(**kw):
    key = tuple(sorted(kw.items()))
    if key not in _NC_CACHE:
        _NC_CACHE[key] = build_gcn(**kw)
    return _NC_CACHE[key]


def kernel(A, x, W1, b1, W2, b2, _trace=False, **build_kw):
    """Full-input entry point: shards internally across 8 NeuronCores."""
    n = A.shape[0]
    R = n // NCORES
    nc = _get_nc(n=n, **build_kw)

    in_maps = []
    for c in range(NCORES):
        in_maps.append({
            "A_blk": np.ascontiguousarray(A[c * R:(c + 1) * R], np.float32),
            "x_blk": np.ascontiguousarray(x[c * R:(c + 1) * R], np.float32),
            "W1": np.ascontiguousarray(W1, np.float32),
            "b1": np.ascontiguousarray(b1, np.float32),
        })
    res = run_bass_kernel_spmd(nc, in_maps, core_ids=list(range(NCORES)),
                               trace=_trace)
    s = np.zeros(HID, np.float32)
    for c in range(NCORES):
        s = s + res.results[c]["s_out"].ravel().astype(np.float32)
    out = s @ np.asarray(W2, np.float32) + np.float32(n) * np.asarray(
        b2, np.float32)
    if _trace:
        kernel.last_results = res
    return out[None, :].astype(np.float32)

